# revision 1
# baseline (speedup 1.0000x reference)
"""MergedQKVParallelLinearWithLora on 8 TRN2 NeuronCores.

Strategy: token-parallel (data-parallel) across the 8 cores — each core
computes 4096 tokens of the full (T=32768, O=3072) output. Per core:

  out^T = W^T-accumulated f32r matmuls (K=2048 contraction over D)
        + lora expand (K=128, bf16)  + lora bias (K=8, bf16)
        + per-channel bias (DVE tensor_scalar_add at PSUM eviction)

The lora shrink s = x @ A^T is computed on-device for all 3 slices and all
8 adapters at once (A stacked to (384, D)), masked per-token by the
adapter one-hot (so tokens with idx==-1 or a different adapter contribute
zero), stored bf16, and consumed as the moving operand of the expand matmul.

Layouts are prepared host-side: x, W, A fed transposed so the contraction
dim D lands on SBUF partitions; output comes back as out^T per core and is
transposed/concatenated on the host.

All SBUF operands are k-tile-granular (128-partition tiles) so DMA/compute
dependencies stay fine-grained: the first matmul issues ~2us in, W tiles for
the next output pass prefetch while the current pass drains, and the x pool
double-buffers token tiles across all four passes.
"""

import numpy as np
import ml_dtypes

import concourse.mybir as mybir
import concourse.tile as tile
from concourse import bacc
from concourse.bass_utils import run_bass_kernel_spmd

T, D, QS, KVS, L, R = 32768, 2048, 2048, 512, 8, 16
O = QS + 2 * KVS          # 3072
NCORES = 8
TC = T // NCORES          # 4096 tokens per core
NT = 512                  # tokens per tile (matmul moving dim)
NKT = D // 128            # 16 contraction k-tiles
NBLK = O // 128           # 24 output-channel blocks
WBLK = 8                  # blocks per W pass (3 passes)

F32 = mybir.dt.float32
F32R = mybir.dt.float32r
BF16 = mybir.dt.bfloat16
BF16NP = ml_dtypes.bfloat16


def build_program(tc_tokens=TC):
    ntt = tc_tokens // NT
    nc = bacc.Bacc(None, target_bir_lowering=False, debug=False)

    xT = nc.dram_tensor("xT", [D, tc_tokens], F32R, kind="ExternalInput")
    wT = nc.dram_tensor("wT", [D, O], F32R, kind="ExternalInput")
    aT = nc.dram_tensor("aT", [D, 3 * 128], F32R, kind="ExternalInput")
    bcomb = nc.dram_tensor("bcomb", [128, O], BF16, kind="ExternalInput")
    biasL = nc.dram_tensor("biasL", [L, O], BF16, kind="ExternalInput")
    bias_arr = nc.dram_tensor("bias_arr", [128, NBLK], F32, kind="ExternalInput")
    maskT = nc.dram_tensor("maskT", [128, tc_tokens], BF16, kind="ExternalInput")
    ohT = nc.dram_tensor("ohT", [L, tc_tokens], BF16, kind="ExternalInput")
    outT = nc.dram_tensor("outT", [O, tc_tokens], F32, kind="ExternalOutput")

    with tile.TileContext(nc) as tc:
        with tc.tile_pool(name="const", bufs=1) as const, \
             tc.tile_pool(name="xp", bufs=8) as xp, \
             tc.tile_pool(name="wp", bufs=4) as wp, \
             tc.tile_pool(name="psm", bufs=8, space="PSUM") as psm, \
             tc.tile_pool(name="op", bufs=4) as op:
            st_all = [const.tile([128, tc_tokens], BF16, tag=f"st{s}", name=f"st{s}")
                      for s in range(3)]
            bc_t = const.tile([128, O], BF16, tag="bc")
            bl_t = const.tile([L, O], BF16, tag="bl")
            oh_t = const.tile([L, tc_tokens], BF16, tag="oh")
            ba_t = const.tile([128, NBLK], F32, tag="ba")

            # quad-batched loads: one DMA covers 4 contraction k-tiles, so the
            # sync sequencer issues 4 descriptors per token tile instead of 16
            def load_x(tt):
                ts = []
                for q in range(NKT // 4):
                    t = xp.tile([128, 4, NT], F32R, tag="x", name=f"x_t{tt}_q{q}")
                    nc.sync.dma_start(
                        out=t[:],
                        in_=xT[:, tt * NT:(tt + 1) * NT].rearrange(
                            "(i p) n -> p i n", p=128)[:, q * 4:(q + 1) * 4, :])
                    ts.append(t)
                return lambda i: ts[i // 4][:, i % 4, :]

            def load_w(p):
                ts = []
                for q in range(NKT // 4):
                    t = wp.tile([128, 4, WBLK * 128], F32R, tag="w",
                                name=f"w_p{p}_q{q}")
                    nc.sync.dma_start(
                        out=t[:],
                        in_=wT[:, p * WBLK * 128:(p + 1) * WBLK * 128].rearrange(
                            "(i p) n -> p i n", p=128)[:, q * 4:(q + 1) * 4, :])
                    ts.append(t)
                return lambda i: ts[i // 4][:, i % 4, :]

            # ---- shrink pass: s~ = mask * (x @ A^T), all tokens, bf16 ----
            # (the main passes' first W tiles prefetch during this pass: the
            # wp pool is open and its slots are free)
            with tc.tile_pool(name="shr", bufs=1) as shr, \
                 tc.tile_pool(name="mkp", bufs=2) as mkp:
                # critical path first: x(tt0) + A feed the very first matmuls
                x_first = load_x(0)
                a_qs = []
                for q in range(NKT // 4):
                    t = shr.tile([128, 4, 384], F32R, tag=f"a{q}", name=f"a_q{q}")
                    nc.sync.dma_start(
                        out=t[:],
                        in_=aT.rearrange("(i p) n -> p i n", p=128)[:, q * 4:(q + 1) * 4, :])
                    a_qs.append(t)
                a_ts = lambda i: a_qs[i // 4][:, i % 4, :]
                nc.gpsimd.dma_start(out=bc_t[:], in_=bcomb[:])
                nc.gpsimd.dma_start(out=bl_t[:], in_=biasL[:])
                nc.gpsimd.dma_start(out=oh_t[:], in_=ohT[:])
                nc.gpsimd.dma_start(out=ba_t[:], in_=bias_arr[:])
                w_next = load_w(0)
                for tt in range(ntt):
                    x_ts = x_first if tt == 0 else load_x(tt)
                    mk_t = mkp.tile([128, NT], BF16, tag="mk")
                    nc.gpsimd.dma_start(
                        out=mk_t[:], in_=maskT[:, tt * NT:(tt + 1) * NT])
                    for s in range(3):
                        ps = psm.tile([128, NT], F32, tag="ps")
                        for i in range(NKT):
                            nc.tensor.matmul(
                                ps[:],
                                a_ts(i)[:, s * 128:(s + 1) * 128],
                                x_ts(i),
                                start=(i == 0), stop=(i == NKT - 1),
                            )
                        nc.vector.tensor_mul(
                            st_all[s][:, tt * NT:(tt + 1) * NT], ps[:], mk_t[:])

            # ---- main passes: 3 x (8 channel-blocks over all tokens) ----
            for p in range(NBLK // WBLK):
                w_ts = w_next
                w_next = load_w(p + 1) if p + 1 < NBLK // WBLK else None
                for tt in range(ntt):
                    x_ts = load_x(tt)
                    # bf16 lora matmuls for all 8 blocks first (one PSUM bank
                    # each: datatype switches only twice per token tile), then
                    # per-block f32r runs with trailing evictions so each
                    # bank frees long before the next tile's bf16 matmul
                    # needs it
                    pss = []
                    for blk in range(WBLK):
                        j = p * WBLK + blk
                        s = 0 if j < QS // 128 else (1 if j < (QS + KVS) // 128 else 2)
                        ps = psm.tile([128, NT], F32, tag="ps", name=f"ps{j}_{tt}")
                        pss.append(ps)
                        nc.tensor.matmul(
                            ps[:],
                            bl_t[:, j * 128:(j + 1) * 128],
                            oh_t[:, tt * NT:(tt + 1) * NT],
                            start=True, stop=False, skip_group_check=True,
                        )
                        nc.tensor.matmul(
                            ps[:],
                            bc_t[:, j * 128:(j + 1) * 128],
                            st_all[s][:, tt * NT:(tt + 1) * NT],
                            start=False, stop=False, skip_group_check=True,
                        )
                    for blk in range(WBLK):
                        j = p * WBLK + blk
                        for i in range(NKT):
                            nc.tensor.matmul(
                                pss[blk][:],
                                w_ts(i)[:, blk * 128:(blk + 1) * 128],
                                x_ts(i),
                                start=False, stop=(i == NKT - 1),
                                skip_group_check=True,
                            )
                        o_t = op.tile([128, NT], F32, tag="o")
                        nc.vector.tensor_scalar_add(o_t[:], pss[blk][:], ba_t[:, j:j + 1])
                        nc.gpsimd.dma_start(
                            out=outT[j * 128:(j + 1) * 128, tt * NT:(tt + 1) * NT],
                            in_=o_t[:],
                        )
    nc.compile()
    return nc


_nc_cache = {}


def _get_program(tc_tokens=TC):
    if tc_tokens not in _nc_cache:
        _nc_cache[tc_tokens] = build_program(tc_tokens)
    return _nc_cache[tc_tokens]


def make_in_maps(x, W_qkv, bias_qkv, lora_a_q, lora_a_k, lora_a_v,
                 lora_b_q, lora_b_k, lora_b_v,
                 lora_bias_q, lora_bias_k, lora_bias_v,
                 token_lora_indices, ncores=NCORES):
    x = np.asarray(x, np.float32)
    idx = np.asarray(token_lora_indices).astype(np.int64)
    tc_tokens = x.shape[0] // ncores

    wT = np.ascontiguousarray(np.asarray(W_qkv, np.float32).T)
    a_stack = np.concatenate([
        np.asarray(lora_a_q, np.float32).reshape(L * R, D),
        np.asarray(lora_a_k, np.float32).reshape(L * R, D),
        np.asarray(lora_a_v, np.float32).reshape(L * R, D)], axis=0)
    aT = np.ascontiguousarray(a_stack.T)
    bcomb = np.concatenate([
        np.asarray(lora_b_q, np.float32).transpose(0, 2, 1).reshape(L * R, QS),
        np.asarray(lora_b_k, np.float32).transpose(0, 2, 1).reshape(L * R, KVS),
        np.asarray(lora_b_v, np.float32).transpose(0, 2, 1).reshape(L * R, KVS)],
        axis=1).astype(BF16NP)
    biasL = np.concatenate([
        np.asarray(lora_bias_q, np.float32),
        np.asarray(lora_bias_k, np.float32),
        np.asarray(lora_bias_v, np.float32)], axis=1).astype(BF16NP)
    bias_arr = np.ascontiguousarray(
        np.asarray(bias_qkv, np.float32).reshape(NBLK, 128).T)
    lane = np.arange(128) // R

    in_maps = []
    for c in range(ncores):
        sl = slice(c * tc_tokens, (c + 1) * tc_tokens)
        idx_c = idx[sl]
        in_maps.append({
            "xT": np.ascontiguousarray(x[sl].T),
            "wT": wT,
            "aT": aT,
            "bcomb": bcomb,
            "biasL": biasL,
            "bias_arr": bias_arr,
            "maskT": (idx_c[None, :] == lane[:, None]).astype(BF16NP),
            "ohT": (idx_c[None, :] == np.arange(L)[:, None]).astype(BF16NP),
        })
    return in_maps, tc_tokens


def kernel(x, W_qkv, bias_qkv, lora_a_q, lora_a_k, lora_a_v,
           lora_b_q, lora_b_k, lora_b_v,
           lora_bias_q, lora_bias_k, lora_bias_v,
           token_lora_indices):
    in_maps, tc_tokens = make_in_maps(
        x, W_qkv, bias_qkv, lora_a_q, lora_a_k, lora_a_v,
        lora_b_q, lora_b_k, lora_b_v,
        lora_bias_q, lora_bias_k, lora_bias_v, token_lora_indices)
    nc = _get_program(tc_tokens)
    res = run_bass_kernel_spmd(nc, in_maps, list(range(NCORES)))
    out = np.empty((T, O), np.float32)
    for c in range(NCORES):
        out[c * tc_tokens:(c + 1) * tc_tokens] = res.results[c]["outT"].T
    return out



# revision 3
# speedup vs baseline: 1.3043x; 1.3043x over previous
"""MergedQKVParallelLinearWithLora on 8 TRN2 NeuronCores.

Token-parallel across the 8 cores: each core computes 4096 tokens of the
full (T=32768, O=3072) output. Per core, per 512-token tile:

  shrink:  s~ = (1/64) * mask * (x8 @ (8*A)^T)   fp8e4 DoubleRow, K=2048
  main:    out^T = lora-expand+bias (one fp8 DoubleRow matmul, K=256:
           subtile0 = s~ rows vs 8*B columns, subtile1 = oh/8 rows vs
           8*lora_bias rows, zero padded)
         + W^T bf16 matmuls (16 k-tiles, K=2048 contraction)
         + per-channel bias (DVE tensor_scalar_add at PSUM eviction)

W (bf16, 96KB/partition) stays fully SBUF-resident, so x streams exactly
once per dtype (bf16 for the base matmul, fp8 pair-layout for the shrink).
All reshapes/transposes/dtype casts are host-side; scales are chosen so
every fp8 operand sits in e4m3's normal range (A,B,bias *8; s~ /8; the
product scales cancel) and the lora delta accumulates into the same PSUM
bank as the base matmul.
"""

import numpy as np
import ml_dtypes

import concourse.mybir as mybir
import concourse.tile as tile
from concourse import bacc
from concourse.bass_utils import run_bass_kernel_spmd

T, D, QS, KVS, L, R = 32768, 2048, 2048, 512, 8, 16
O = QS + 2 * KVS          # 3072
NCORES = 8
TC = T // NCORES          # 4096 tokens per core
NT = 512                  # tokens per tile (matmul moving dim)
NTT = TC // NT            # 8 token tiles
NKT = D // 128            # 16 contraction k-tiles
NBLK = O // 128           # 24 output-channel blocks
WBLK = 8                  # blocks per sub-pass (3 sub-passes)
NPASS = NBLK // WBLK

F32 = mybir.dt.float32
BF16 = mybir.dt.bfloat16
FP8 = mybir.dt.float8e4
DR = mybir.MatmulPerfMode.DoubleRow
BF16NP = ml_dtypes.bfloat16
E4NP = ml_dtypes.float8_e4m3


def build_program(tc_tokens=TC):
    ntt = tc_tokens // NT
    nc = bacc.Bacc(None, target_bir_lowering=False, debug=False)

    x8d = nc.dram_tensor("x8d", [ntt, 128, NKT, NT], FP8, kind="ExternalInput")
    xbd = nc.dram_tensor("xbd", [ntt, 128, NKT, NT], BF16, kind="ExternalInput")
    wd = nc.dram_tensor("wd", [NPASS, 128, NKT, WBLK * 128], BF16,
                        kind="ExternalInput")
    a8d = nc.dram_tensor("a8d", [128, NKT, 384], FP8, kind="ExternalInput")
    bcld = nc.dram_tensor("bcld", [128, 2, O], FP8, kind="ExternalInput")
    m8d = nc.dram_tensor("m8d", [128, tc_tokens], FP8, kind="ExternalInput")
    oh8d = nc.dram_tensor("oh8d", [8, tc_tokens], FP8, kind="ExternalInput")
    bad = nc.dram_tensor("bad", [128, NBLK], F32, kind="ExternalInput")
    outT = nc.dram_tensor("outT", [O, tc_tokens], F32, kind="ExternalOutput")

    def slice_of(j):
        return 0 if j < QS // 128 else (1 if j < (QS + KVS) // 128 else 2)

    with tile.TileContext(nc) as tc:
        with tc.tile_pool(name="const", bufs=1) as const, \
             tc.tile_pool(name="x8p", bufs=2) as x8p, \
             tc.tile_pool(name="xbp", bufs=2) as xbp, \
             tc.tile_pool(name="psm", bufs=8, space="PSUM") as psm, \
             tc.tile_pool(name="op", bufs=4) as op:
            a8_t = const.tile([128, NKT, 384], FP8, tag="a8")
            bcl_t = const.tile([128, 2, O], FP8, tag="bcl")
            m8_t = const.tile([128, tc_tokens], FP8, tag="m8")
            ba_t = const.tile([128, NBLK], F32, tag="ba")
            st_all = [const.tile([128, 2, tc_tokens], FP8, tag=f"st{s}",
                                 name=f"st{s}") for s in range(3)]
            w_all = [const.tile([128, NKT, WBLK * 128], BF16, tag=f"w{p}",
                                name=f"w{p}") for p in range(NPASS)]

            def load_x8(tt):
                t = x8p.tile([128, NKT, NT], FP8, tag="x8", name=f"x8_{tt}")
                nc.sync.dma_start(out=t[:], in_=x8d[tt])
                return t

            def load_xb(tt):
                t = xbp.tile([128, NKT, NT], BF16, tag="xb", name=f"xb_{tt}")
                nc.sync.dma_start(out=t[:], in_=xbd[tt])
                return t

            # critical path first: tile-0 x8 + A8 feed the first matmuls
            x8_t = load_x8(0)
            nc.sync.dma_start(out=a8_t[:], in_=a8d[:])
            nc.gpsimd.dma_start(out=m8_t[:], in_=m8d[:])
            xb_t = load_xb(0)
            for s in range(3):
                nc.any.memset(st_all[s][:, 1, :], 0)
                nc.gpsimd.dma_start(out=st_all[s][0:8, 1, :], in_=oh8d[:])
            nc.gpsimd.dma_start(out=bcl_t[:], in_=bcld[:])
            nc.gpsimd.dma_start(out=ba_t[:], in_=bad[:])
            for p in range(NPASS):
                nc.sync.dma_start(out=w_all[p][:], in_=wd[p])

            for tt in range(ntt):
                tsl = slice(tt * NT, (tt + 1) * NT)
                # ---- shrink: fp8 DoubleRow, all 3 slices ----
                for s in range(3):
                    ps = psm.tile([128, NT], F32, tag="ps", name=f"shr{s}_{tt}")
                    for k in range(NKT // 2):
                        nc.tensor.matmul(
                            ps[:],
                            a8_t[:, 2 * k:2 * k + 2, s * 128:(s + 1) * 128],
                            x8_t[:, 2 * k:2 * k + 2, :],
                            start=(k == 0), stop=(k == NKT // 2 - 1),
                            perf_mode=DR, skip_group_check=True,
                        )
                    nc.vector.tensor_mul(st_all[s][:, 0, tsl], ps[:], m8_t[:, tsl])

                # prefetch next token tile while mains chew
                x8_next = load_x8(tt + 1) if tt + 1 < ntt else None
                xb_next = load_xb(tt + 1) if tt + 1 < ntt else None

                # ---- main: 3 sub-passes of 8 channel blocks ----
                for p in range(NPASS):
                    pss = []
                    for blk in range(WBLK):
                        j = p * WBLK + blk
                        s = slice_of(j)
                        ps = psm.tile([128, NT], F32, tag="ps", name=f"ps{j}_{tt}")
                        pss.append(ps)
                        nc.tensor.matmul(
                            ps[:],
                            bcl_t[:, :, j * 128:(j + 1) * 128],
                            st_all[s][:, :, tsl],
                            start=True, stop=False,
                            perf_mode=DR, skip_group_check=True,
                        )
                    for blk in range(WBLK):
                        j = p * WBLK + blk
                        for i in range(NKT):
                            nc.tensor.matmul(
                                pss[blk][:],
                                w_all[p][:, i, blk * 128:(blk + 1) * 128],
                                xb_t[:, i, :],
                                start=False, stop=(i == NKT - 1),
                                skip_group_check=True,
                            )
                        o_t = op.tile([128, NT], F32, tag="o")
                        nc.vector.tensor_scalar_add(o_t[:], pss[blk][:],
                                                    ba_t[:, j:j + 1])
                        nc.gpsimd.dma_start(
                            out=outT[j * 128:(j + 1) * 128, tsl], in_=o_t[:])
                x8_t, xb_t = x8_next, xb_next
    nc.compile()
    return nc


_nc_cache = {}


def _get_program(tc_tokens=TC):
    if tc_tokens not in _nc_cache:
        _nc_cache[tc_tokens] = build_program(tc_tokens)
    return _nc_cache[tc_tokens]


def make_in_maps(x, W_qkv, bias_qkv, lora_a_q, lora_a_k, lora_a_v,
                 lora_b_q, lora_b_k, lora_b_v,
                 lora_bias_q, lora_bias_k, lora_bias_v,
                 token_lora_indices, ncores=NCORES):
    x = np.asarray(x, np.float32)
    idx = np.asarray(token_lora_indices).astype(np.int64)
    tc_tokens = x.shape[0] // ncores
    ntt = tc_tokens // NT

    # W: (NPASS, 128, NKT, WBLK*128); [pi, p, i, m] = W_qkv[pi*1024+m, i*128+p]
    wd = np.ascontiguousarray(
        np.asarray(W_qkv, np.float32).reshape(NPASS, WBLK * 128, NKT, 128)
        .transpose(0, 3, 2, 1)).astype(BF16NP)
    # A8: (128, NKT, 384); [p, i, m] = 8 * A_stack[m, i*128+p]
    a_stack = np.concatenate([
        np.asarray(lora_a_q, np.float32).reshape(L * R, D),
        np.asarray(lora_a_k, np.float32).reshape(L * R, D),
        np.asarray(lora_a_v, np.float32).reshape(L * R, D)], axis=0)
    a8d = np.ascontiguousarray(
        (a_stack * 8.0).reshape(384, NKT, 128).transpose(2, 1, 0)).astype(E4NP)
    # bcl: (128, 2, O); [:,0,:] = 8*B^T rows (l*R+r), [:8,1,:] = 8*lora_bias
    bcomb = np.concatenate([
        np.asarray(lora_b_q, np.float32).transpose(0, 2, 1).reshape(L * R, QS),
        np.asarray(lora_b_k, np.float32).transpose(0, 2, 1).reshape(L * R, KVS),
        np.asarray(lora_b_v, np.float32).transpose(0, 2, 1).reshape(L * R, KVS)],
        axis=1)
    biasL = np.concatenate([
        np.asarray(lora_bias_q, np.float32),
        np.asarray(lora_bias_k, np.float32),
        np.asarray(lora_bias_v, np.float32)], axis=1)
    bcld = np.zeros((128, 2, O), np.float32)
    bcld[:, 0, :] = bcomb * 8.0
    bcld[:8, 1, :] = biasL * 8.0
    bcld = bcld.astype(E4NP)
    bad = np.ascontiguousarray(
        np.asarray(bias_qkv, np.float32).reshape(NBLK, 128).T)
    lane = np.arange(128) // R

    in_maps = []
    for c in range(ncores):
        sl = slice(c * tc_tokens, (c + 1) * tc_tokens)
        xc = x[sl]
        # (ntt, 128, NKT, NT); [tt, p, i, n] = x[tt*NT+n, i*128+p]
        xr = np.ascontiguousarray(
            xc.reshape(ntt, NT, NKT, 128).transpose(0, 3, 2, 1))
        idx_c = idx[sl]
        in_maps.append({
            "x8d": xr.astype(E4NP),
            "xbd": xr.astype(BF16NP),
            "wd": wd,
            "a8d": a8d,
            "bcld": bcld,
            "m8d": np.where(idx_c[None, :] == lane[:, None],
                            np.float32(1 / 64), np.float32(0)).astype(E4NP),
            "oh8d": np.where(idx_c[None, :] == np.arange(L)[:, None],
                             np.float32(0.125), np.float32(0)).astype(E4NP),
            "bad": bad,
        })
    return in_maps, tc_tokens


def kernel(x, W_qkv, bias_qkv, lora_a_q, lora_a_k, lora_a_v,
           lora_b_q, lora_b_k, lora_b_v,
           lora_bias_q, lora_bias_k, lora_bias_v,
           token_lora_indices):
    in_maps, tc_tokens = make_in_maps(
        x, W_qkv, bias_qkv, lora_a_q, lora_a_k, lora_a_v,
        lora_b_q, lora_b_k, lora_b_v,
        lora_bias_q, lora_bias_k, lora_bias_v, token_lora_indices)
    nc = _get_program(tc_tokens)
    res = run_bass_kernel_spmd(nc, in_maps, list(range(NCORES)))
    out = np.empty((T, O), np.float32)
    for c in range(NCORES):
        out[c * tc_tokens:(c + 1) * tc_tokens] = res.results[c]["outT"].T
    return out
